# revision 11
# baseline (speedup 1.0000x reference)
"""STEBitLinear Trainium2 kernel.

y[b,s,o] = sum_i x[b,s,i] * sign(w[o,i]) * scale[o, i//128]

Strategy: data-parallel over the flattened (b,s) dim across 8 NeuronCores
(weights/scales replicated, no collectives). Per core:
  - cast x shard to bf16 and transpose it into a resident SBUF x^T
  - per 512-wide out-feature tile: build w_eff^T = (sign*scale)^T in bf16
    (fused cast+scale via per-partition tensor_scalar, then transpose)
  - 128x128x512 bf16 matmuls accumulating over K=4096 in PSUM (fp32)

All transposes are NORMAL bf16 matmuls against a 128x128 identity
(out = chunk.T @ I): unlike PE transpose-mode these run at warm-matmul
speed and keep the HAM clock gate engaged. The o-tile pipeline is
software-pipelined at emission: the w^T build for tile t+1 is emitted
before tile t's matmul loop, so its PE transposes slot in right after
tile t's matmuls and its DVE scale ops run during them. PSUM result
evacuation runs on the otherwise-idle Scalar (ACT) engine so it never
head-of-line blocks DVE's scale pipeline.
"""

import sys

for _p in ("/opt/trn_rl_repo", "/opt/pypackages"):
    if _p not in sys.path:
        sys.path.append(_p)

import numpy as np

import concourse.bacc as bacc
import concourse.mybir as mybir
from concourse.bass_utils import run_bass_kernel_spmd
from concourse.masks import make_identity
from concourse.tile import TileContext

N_CORES = 8
B, S, IN_F, OUT_F = 4, 2048, 4096, 4096
GROUP = 128
M_FULL = B * S  # 8192


def build_program(M=M_FULL // N_CORES, K=IN_F, N=OUT_F, n_tile=512, ld=1024):
    """Emit the per-core Bass program (SPMD: same program on all cores)."""
    P = 128
    KT = K // P            # k tiles (contraction, partition dim)
    MT = M // P            # m tiles
    NT = N // n_tile       # out-feature tiles
    NSUB = n_tile // P     # 128-wide o sub-blocks per o tile
    LC = K // ld           # load chunks per row-block
    LG = ld // P           # 128-wide groups per load chunk
    G = K // GROUP         # scale groups along in_features
    NB = N // P            # o blocks of 128
    bf16 = mybir.dt.bfloat16
    f32 = mybir.dt.float32

    nc = bacc.Bacc("TRN2", target_bir_lowering=False, debug=False)
    x_d = nc.dram_tensor("x", [M, K], f32, kind="ExternalInput").ap()
    w_d = nc.dram_tensor("sw", [N, K], f32, kind="ExternalInput").ap()
    sc_d = nc.dram_tensor("sc", [N, G], f32, kind="ExternalInput").ap()
    y_d = nc.dram_tensor("y", [M, N], f32, kind="ExternalOutput").ap()

    with TileContext(nc) as tc:
        with (
            tc.tile_pool(name="consts", bufs=1) as consts,
            tc.tile_pool(name="xt_pool", bufs=1) as xt_pool,
            tc.tile_pool(name="wt_pool", bufs=2) as wt_pool,
            tc.tile_pool(name="load", bufs=3) as load_pool,
            tc.tile_pool(name="stage", bufs=4) as stage_pool,
            tc.tile_pool(name="ysb", bufs=2) as y_pool,
            tc.tile_pool(name="wdram", bufs=3, space="DRAM") as wdram_pool,
            tc.tile_pool(name="pst", bufs=4, space="PSUM") as psum_t,
            tc.tile_pool(name="psa", bufs=3, space="PSUM") as psum_a,
        ):
            ident = consts.tile([P, P], bf16)
            make_identity(nc, ident)

            # scales resident: sc_sb[p, ob*G + g] = scales[ob*128 + p, g]
            # (gpsimd/SWDGE ring: keeps the HWDGE ring free for x/w loads)
            sc_sb = consts.tile([P, NB * G], f32)
            for ob in range(NB):
                nc.gpsimd.dma_start(
                    out=sc_sb[:, ob * G:(ob + 1) * G],
                    in_=sc_d[ob * P:(ob + 1) * P, :],
                )

            def mm_transpose(dst_v, src, k0, col0):
                """dst_v[:, k0+c, col0:col0+128] = src[:, c*128:(c+1)*128].T
                for c in range(LG), via normal matmuls against identity."""
                for h in range(LG // 4):
                    pt = psum_t.tile([P, 512], f32, tag="pt")
                    for g in range(4):
                        c = h * 4 + g
                        nc.tensor.matmul(
                            pt[:, g * P:(g + 1) * P],
                            src[:, c * P:(c + 1) * P],
                            ident,
                            start=True,
                            stop=True,
                        )
                    pt_v = pt.rearrange("p (g c) -> p g c", g=4)
                    nc.vector.tensor_copy(
                        out=dst_v[:, k0 + h * 4:k0 + h * 4 + 4, col0:col0 + P],
                        in_=pt_v,
                    )

            # ---- phase 0: x^T resident (bf16), [P, KT * M] ----
            xT = xt_pool.tile([P, KT * M], bf16)
            xT_v = xT.rearrange("p (k m) -> p k m", k=KT)
            for mt in range(MT):
                for lc in range(LC):
                    xin = load_pool.tile([P, ld], f32, tag="xload")
                    nc.sync.dma_start(
                        out=xin,
                        in_=x_d[mt * P:(mt + 1) * P, lc * ld:(lc + 1) * ld],
                    )
                    xbf = stage_pool.tile([P, ld], bf16, tag="xcast")
                    nc.vector.tensor_copy(out=xbf, in_=xin)
                    mm_transpose(xT_v, xbf, lc * LG, mt * P)

            # ---- main loop over out-feature tiles (software-pipelined) ----
            # Half of each o tile's w^T build stays on the PE (transpose
            # matmuls); the other half bounces w_eff through DRAM and loads
            # back via the XBAR dma transpose, using idle DMA capacity.
            LCX = LC // 2  # load chunks routed via XBAR

            def build_wT(ot):
                """w_eff^T tiles for o tile `ot`: load, scale (DVE),
                transpose (PE or XBAR), gather into [P, KT * n_tile] bf16."""
                wT = wt_pool.tile([P, KT * n_tile], bf16, tag="wt")
                wT_v = wT.rearrange("p (k o) -> p k o", k=KT)
                weff = wdram_pool.tile([n_tile, LCX * ld], bf16, tag="weff")
                for j in range(NSUB):
                    ob = ot * NSUB + j
                    for lc in range(LC):
                        win = load_pool.tile([P, ld], f32, tag="wload")
                        nc.sync.dma_start(
                            out=win,
                            in_=w_d[ob * P:(ob + 1) * P, lc * ld:(lc + 1) * ld],
                        )
                        wst = stage_pool.tile([P, ld], bf16, tag="wstage")
                        for g in range(LG):
                            gk = lc * LG + g
                            nc.vector.tensor_scalar_mul(
                                out=wst[:, g * P:(g + 1) * P],
                                in0=win[:, g * P:(g + 1) * P],
                                scalar1=sc_sb[:, ob * G + gk:ob * G + gk + 1],
                            )
                        if lc < LCX:
                            nc.sync.dma_start(
                                out=weff[j * P:(j + 1) * P, lc * ld:(lc + 1) * ld],
                                in_=wst,
                            )
                        else:
                            mm_transpose(wT_v, wst, lc * LG, j * P)
                for kk in range(LCX * LG):
                    nc.sync.dma_start_transpose(
                        out=wT_v[:, kk, :],
                        in_=weff[:, kk * P:(kk + 1) * P],
                    )
                return wT_v

            wT_cur = build_wT(0)
            wT_nxt = build_wT(1) if NT > 1 else None
            for ot in range(NT):
                wT_v = wT_cur
                for mt in range(MT):
                    acc = psum_a.tile([P, n_tile], f32, tag="acc")
                    for k in range(KT):
                        nc.tensor.matmul(
                            acc,
                            xT_v[:, k, mt * P:(mt + 1) * P],
                            wT_v[:, k],
                            start=(k == 0),
                            stop=(k == KT - 1),
                        )
                    ysb = y_pool.tile([P, n_tile], f32, tag="ysb")
                    nc.scalar.copy(out=ysb, in_=acc)
                    nc.sync.dma_start(
                        out=y_d[mt * P:(mt + 1) * P, ot * n_tile:(ot + 1) * n_tile],
                        in_=ysb,
                    )
                wT_cur = wT_nxt
                if ot + 2 < NT:
                    wT_nxt = build_wT(ot + 2)

    nc.compile()
    return nc


_nc_cache = {}


def _get_nc(key, **kw):
    if key not in _nc_cache:
        _nc_cache[key] = build_program(**kw)
    return _nc_cache[key]


def kernel(x: np.ndarray, sign_weights: np.ndarray, scales: np.ndarray) -> np.ndarray:
    nc = _get_nc("full")
    M_SH = M_FULL // N_CORES
    xf = np.ascontiguousarray(x.reshape(M_FULL, IN_F).astype(np.float32, copy=False))
    sw = np.ascontiguousarray(sign_weights.astype(np.float32, copy=False))
    sc = np.ascontiguousarray(scales.reshape(OUT_F, IN_F // GROUP))
    in_maps = [
        {"x": xf[c * M_SH:(c + 1) * M_SH], "sw": sw, "sc": sc}
        for c in range(N_CORES)
    ]
    res = run_bass_kernel_spmd(nc, in_maps, core_ids=list(range(N_CORES)))
    y = np.concatenate([res.results[c]["y"] for c in range(N_CORES)], axis=0)
    return y.reshape(B, S, OUT_F)


# revision 14
# speedup vs baseline: 1.1335x; 1.1335x over previous
"""STEBitLinear Trainium2 kernel.

y[b,s,o] = sum_i x[b,s,i] * sign(w[o,i]) * scale[o, i//128]

Strategy: data-parallel over the flattened (b,s) dim across 8 NeuronCores
(weights/scales replicated, no collectives). Per core:
  - cast x shard to bf16 and transpose it into a resident SBUF x^T
  - per 512-wide out-feature tile: build w_eff^T = (sign*scale)^T in bf16
    (fused cast+scale via per-partition tensor_scalar, then transpose)
  - 128x128x512 bf16 matmuls accumulating over K=4096 in PSUM (fp32)

All transposes are NORMAL bf16 matmuls against a 128x128 identity
(out = chunk.T @ I): unlike PE transpose-mode these run at warm-matmul
speed and keep the HAM clock gate engaged. The o-tile pipeline is
software-pipelined at emission: the w^T build for tile t+1 is emitted
before tile t's matmul loop, so its PE transposes slot in right after
tile t's matmuls and its DVE scale ops run during them. PSUM result
evacuation runs on the otherwise-idle Scalar (ACT) engine so it never
head-of-line blocks DVE's scale pipeline.
"""

import sys

for _p in ("/opt/trn_rl_repo", "/opt/pypackages"):
    if _p not in sys.path:
        sys.path.append(_p)

import numpy as np

import concourse.bacc as bacc
import concourse.mybir as mybir
from concourse.bass_utils import run_bass_kernel_spmd
from concourse.masks import make_identity
from concourse.tile import TileContext

N_CORES = 8
B, S, IN_F, OUT_F = 4, 2048, 4096, 4096
GROUP = 128
M_FULL = B * S  # 8192


def build_program(M=M_FULL // N_CORES, K=IN_F, N=OUT_F, n_tile=512, ld=1024):
    """Emit the per-core Bass program (SPMD: same program on all cores)."""
    P = 128
    KT = K // P            # k tiles (contraction, partition dim)
    MT = M // P            # m tiles
    NT = N // n_tile       # out-feature tiles
    NSUB = n_tile // P     # 128-wide o sub-blocks per o tile
    LC = K // ld           # load chunks per row-block
    LG = ld // P           # 128-wide groups per load chunk
    G = K // GROUP         # scale groups along in_features
    NB = N // P            # o blocks of 128
    bf16 = mybir.dt.bfloat16
    f32 = mybir.dt.float32

    nc = bacc.Bacc("TRN2", target_bir_lowering=False, debug=False)
    x_d = nc.dram_tensor("x", [M, K], f32, kind="ExternalInput").ap()
    w_d = nc.dram_tensor("sw", [N, K], f32, kind="ExternalInput").ap()
    sc_d = nc.dram_tensor("sc", [N, G], f32, kind="ExternalInput").ap()
    y_d = nc.dram_tensor("y", [M, N], f32, kind="ExternalOutput").ap()

    with TileContext(nc) as tc:
        with (
            tc.tile_pool(name="consts", bufs=1) as consts,
            tc.tile_pool(name="xt_pool", bufs=1) as xt_pool,
            tc.tile_pool(name="wt_pool", bufs=2) as wt_pool,
            tc.tile_pool(name="load", bufs=3) as load_pool,
            tc.tile_pool(name="stage", bufs=4) as stage_pool,
            tc.tile_pool(name="ysb", bufs=2) as y_pool,
            tc.tile_pool(name="pst", bufs=4, space="PSUM") as psum_t,
            tc.tile_pool(name="psa", bufs=3, space="PSUM") as psum_a,
        ):
            ident = consts.tile([P, P], bf16)
            make_identity(nc, ident)

            # scales resident: sc_sb[p, ob*G + g] = scales[ob*128 + p, g]
            # (gpsimd/SWDGE ring: keeps the HWDGE ring free for x/w loads)
            sc_sb = consts.tile([P, NB * G], f32)
            for ob in range(NB):
                nc.gpsimd.dma_start(
                    out=sc_sb[:, ob * G:(ob + 1) * G],
                    in_=sc_d[ob * P:(ob + 1) * P, :],
                )

            cb_flip = [0]

            def mm_transpose(dst_v, src, k0, col0):
                """dst_v[:, k0+c, col0:col0+128] = src[:, c*128:(c+1)*128].T
                for c in range(LG), via normal matmuls against identity.
                PSUM copybacks alternate DVE/ACT so neither engine rate-limits
                the transpose bursts."""
                for h in range(LG // 4):
                    pt = psum_t.tile([P, 512], f32, tag="pt")
                    for g in range(4):
                        c = h * 4 + g
                        nc.tensor.matmul(
                            pt[:, g * P:(g + 1) * P],
                            src[:, c * P:(c + 1) * P],
                            ident,
                            start=True,
                            stop=True,
                        )
                    pt_v = pt.rearrange("p (g c) -> p g c", g=4)
                    dst = dst_v[:, k0 + h * 4:k0 + h * 4 + 4, col0:col0 + P]
                    cb_flip[0] ^= 1
                    if cb_flip[0]:
                        nc.vector.tensor_copy(out=dst, in_=pt_v)
                    else:
                        nc.scalar.copy(out=dst, in_=pt_v)

            # ---- phase 0: x^T resident (bf16), [P, KT * M] ----
            xT = xt_pool.tile([P, KT * M], bf16)
            xT_v = xT.rearrange("p (k m) -> p k m", k=KT)
            for mt in range(MT):
                for lc in range(LC):
                    xin = load_pool.tile([P, ld], f32, tag="xload")
                    nc.sync.dma_start(
                        out=xin,
                        in_=x_d[mt * P:(mt + 1) * P, lc * ld:(lc + 1) * ld],
                    )
                    xbf = stage_pool.tile([P, ld], bf16, tag="xcast")
                    nc.vector.tensor_copy(out=xbf, in_=xin)
                    mm_transpose(xT_v, xbf, lc * LG, mt * P)

            # ---- main loop over out-feature tiles (software-pipelined) ----
            def build_wT(ot):
                """w_eff^T tiles for o tile `ot`: load, scale (DVE),
                transpose (PE), gather into a [P, KT * n_tile] bf16 tile."""
                wT = wt_pool.tile([P, KT * n_tile], bf16, tag="wt")
                wT_v = wT.rearrange("p (k o) -> p k o", k=KT)
                for j in range(NSUB):
                    ob = ot * NSUB + j
                    for lc in range(LC):
                        win = load_pool.tile([P, ld], f32, tag="wload")
                        nc.sync.dma_start(
                            out=win,
                            in_=w_d[ob * P:(ob + 1) * P, lc * ld:(lc + 1) * ld],
                        )
                        wst = stage_pool.tile([P, ld], bf16, tag="wstage")
                        for g in range(LG):
                            gk = lc * LG + g
                            nc.vector.tensor_scalar_mul(
                                out=wst[:, g * P:(g + 1) * P],
                                in0=win[:, g * P:(g + 1) * P],
                                scalar1=sc_sb[:, ob * G + gk:ob * G + gk + 1],
                            )
                        mm_transpose(wT_v, wst, lc * LG, j * P)
                return wT_v

            wT_cur = build_wT(0)
            wT_nxt = build_wT(1) if NT > 1 else None
            for ot in range(NT):
                wT_v = wT_cur
                for mt in range(MT):
                    acc = psum_a.tile([P, n_tile], f32, tag="acc")
                    for k in range(KT):
                        nc.tensor.matmul(
                            acc,
                            xT_v[:, k, mt * P:(mt + 1) * P],
                            wT_v[:, k],
                            start=(k == 0),
                            stop=(k == KT - 1),
                        )
                    ysb = y_pool.tile([P, n_tile], f32, tag="ysb")
                    nc.scalar.copy(out=ysb, in_=acc)
                    nc.sync.dma_start(
                        out=y_d[mt * P:(mt + 1) * P, ot * n_tile:(ot + 1) * n_tile],
                        in_=ysb,
                    )
                wT_cur = wT_nxt
                if ot + 2 < NT:
                    wT_nxt = build_wT(ot + 2)

    nc.compile()
    return nc


_nc_cache = {}


def _get_nc(key, **kw):
    if key not in _nc_cache:
        _nc_cache[key] = build_program(**kw)
    return _nc_cache[key]


def kernel(x: np.ndarray, sign_weights: np.ndarray, scales: np.ndarray) -> np.ndarray:
    nc = _get_nc("full")
    M_SH = M_FULL // N_CORES
    xf = np.ascontiguousarray(x.reshape(M_FULL, IN_F).astype(np.float32, copy=False))
    sw = np.ascontiguousarray(sign_weights.astype(np.float32, copy=False))
    sc = np.ascontiguousarray(scales.reshape(OUT_F, IN_F // GROUP))
    in_maps = [
        {"x": xf[c * M_SH:(c + 1) * M_SH], "sw": sw, "sc": sc}
        for c in range(N_CORES)
    ]
    res = run_bass_kernel_spmd(nc, in_maps, core_ids=list(range(N_CORES)))
    y = np.concatenate([res.results[c]["y"] for c in range(N_CORES)], axis=0)
    return y.reshape(B, S, OUT_F)


# revision 15
# speedup vs baseline: 1.1990x; 1.0578x over previous
"""STEBitLinear Trainium2 kernel.

y[b,s,o] = sum_i x[b,s,i] * sign(w[o,i]) * scale[o, i//128]

Strategy: data-parallel over the flattened (b,s) dim across 8 NeuronCores
(weights/scales replicated, no collectives). Per core:
  - cast x shard to bf16 and transpose it into a resident SBUF x^T
  - per 512-wide out-feature tile: build w_eff^T = (sign*scale)^T in bf16
    (fused cast+scale via per-partition tensor_scalar, then transpose)
  - 128x128x512 bf16 matmuls accumulating over K=4096 in PSUM (fp32)

All transposes are NORMAL bf16 matmuls against a 128x128 identity
(out = chunk.T @ I): unlike PE transpose-mode these run at warm-matmul
speed and keep the HAM clock gate engaged. The o-tile pipeline is
software-pipelined at emission: the w^T build for tile t+1 is emitted
before tile t's matmul loop, so its PE transposes slot in right after
tile t's matmuls and its DVE scale ops run during them. PSUM result
evacuation runs on the otherwise-idle Scalar (ACT) engine so it never
head-of-line blocks DVE's scale pipeline.
"""

import sys

for _p in ("/opt/trn_rl_repo", "/opt/pypackages"):
    if _p not in sys.path:
        sys.path.append(_p)

import numpy as np

import concourse.bacc as bacc
import concourse.mybir as mybir
from concourse.bass_utils import run_bass_kernel_spmd
from concourse.masks import make_identity
from concourse.tile import TileContext

N_CORES = 8
B, S, IN_F, OUT_F = 4, 2048, 4096, 4096
GROUP = 128
M_FULL = B * S  # 8192


def build_program(M=M_FULL // N_CORES, K=IN_F, N=OUT_F, n_tile=512, ld=1024):
    """Emit the per-core Bass program (SPMD: same program on all cores)."""
    P = 128
    KT = K // P            # k tiles (contraction, partition dim)
    MT = M // P            # m tiles
    NT = N // n_tile       # out-feature tiles
    NSUB = n_tile // P     # 128-wide o sub-blocks per o tile
    LC = K // ld           # load chunks per row-block
    LG = ld // P           # 128-wide groups per load chunk
    G = K // GROUP         # scale groups along in_features
    NB = N // P            # o blocks of 128
    bf16 = mybir.dt.bfloat16
    f32 = mybir.dt.float32

    nc = bacc.Bacc("TRN2", target_bir_lowering=False, debug=False)
    x_d = nc.dram_tensor("x", [M, K], f32, kind="ExternalInput").ap()
    w_d = nc.dram_tensor("sw", [N, K], f32, kind="ExternalInput").ap()
    sc_d = nc.dram_tensor("sc", [N, G], f32, kind="ExternalInput").ap()
    y_d = nc.dram_tensor("y", [M, N], f32, kind="ExternalOutput").ap()

    with TileContext(nc) as tc:
        with (
            tc.tile_pool(name="consts", bufs=1) as consts,
            tc.tile_pool(name="xt_pool", bufs=1) as xt_pool,
            tc.tile_pool(name="wt_pool", bufs=2) as wt_pool,
            tc.tile_pool(name="load", bufs=3) as load_pool,
            tc.tile_pool(name="stage", bufs=4) as stage_pool,
            tc.tile_pool(name="ysb", bufs=2) as y_pool,
            tc.tile_pool(name="pst", bufs=5, space="PSUM") as psum_t,
            tc.tile_pool(name="psa", bufs=3, space="PSUM") as psum_a,
        ):
            ident = consts.tile([P, P], bf16)
            make_identity(nc, ident)

            # scales resident: sc_sb[p, ob*G + g] = scales[ob*128 + p, g]
            # (gpsimd/SWDGE ring: keeps the HWDGE ring free for x/w loads)
            sc_sb = consts.tile([P, NB * G], f32)
            for ob in range(NB):
                nc.gpsimd.dma_start(
                    out=sc_sb[:, ob * G:(ob + 1) * G],
                    in_=sc_d[ob * P:(ob + 1) * P, :],
                )

            cb_flip = [0]

            def mm_transpose(dst_v, src, k0, col0):
                """dst_v[:, k0+c, col0:col0+128] = src[:, c*128:(c+1)*128].T
                for c in range(LG), via normal matmuls against identity.
                PSUM copybacks alternate DVE/ACT so neither engine rate-limits
                the transpose bursts."""
                for h in range(LG // 4):
                    pt = psum_t.tile([P, 512], f32, tag="pt")
                    for g in range(4):
                        c = h * 4 + g
                        nc.tensor.matmul(
                            pt[:, g * P:(g + 1) * P],
                            src[:, c * P:(c + 1) * P],
                            ident,
                            start=True,
                            stop=True,
                        )
                    pt_v = pt.rearrange("p (g c) -> p g c", g=4)
                    nc.vector.tensor_copy(
                        out=dst_v[:, k0 + h * 4:k0 + h * 4 + 4, col0:col0 + P],
                        in_=pt_v,
                    )

            # ---- phase 0: x^T resident (bf16), [P, KT * M] ----
            xT = xt_pool.tile([P, KT * M], bf16)
            xT_v = xT.rearrange("p (k m) -> p k m", k=KT)
            for mt in range(MT):
                for lc in range(LC):
                    xin = load_pool.tile([P, ld], f32, tag="xload")
                    nc.sync.dma_start(
                        out=xin,
                        in_=x_d[mt * P:(mt + 1) * P, lc * ld:(lc + 1) * ld],
                    )
                    xbf = stage_pool.tile([P, ld], bf16, tag="xcast")
                    nc.vector.tensor_copy(out=xbf, in_=xin)
                    mm_transpose(xT_v, xbf, lc * LG, mt * P)

            # ---- main loop over out-feature tiles (software-pipelined) ----
            def build_wT(ot):
                """w_eff^T tiles for o tile `ot`: load, scale (DVE),
                transpose (PE), gather into a [P, KT * n_tile] bf16 tile."""
                wT = wt_pool.tile([P, KT * n_tile], bf16, tag="wt")
                wT_v = wT.rearrange("p (k o) -> p k o", k=KT)
                for j in range(NSUB):
                    ob = ot * NSUB + j
                    for lc in range(LC):
                        win = load_pool.tile([P, ld], f32, tag="wload")
                        nc.sync.dma_start(
                            out=win,
                            in_=w_d[ob * P:(ob + 1) * P, lc * ld:(lc + 1) * ld],
                        )
                        wst = stage_pool.tile([P, ld], bf16, tag="wstage")
                        for g in range(LG):
                            gk = lc * LG + g
                            nc.vector.tensor_scalar_mul(
                                out=wst[:, g * P:(g + 1) * P],
                                in0=win[:, g * P:(g + 1) * P],
                                scalar1=sc_sb[:, ob * G + gk:ob * G + gk + 1],
                            )
                        mm_transpose(wT_v, wst, lc * LG, j * P)
                return wT_v

            wT_cur = build_wT(0)
            wT_nxt = build_wT(1) if NT > 1 else None
            for ot in range(NT):
                wT_v = wT_cur
                for mt in range(MT):
                    acc = psum_a.tile([P, n_tile], f32, tag="acc")
                    for k in range(KT):
                        nc.tensor.matmul(
                            acc,
                            xT_v[:, k, mt * P:(mt + 1) * P],
                            wT_v[:, k],
                            start=(k == 0),
                            stop=(k == KT - 1),
                        )
                    ysb = y_pool.tile([P, n_tile], f32, tag="ysb")
                    nc.scalar.copy(out=ysb, in_=acc)
                    nc.sync.dma_start(
                        out=y_d[mt * P:(mt + 1) * P, ot * n_tile:(ot + 1) * n_tile],
                        in_=ysb,
                    )
                wT_cur = wT_nxt
                if ot + 2 < NT:
                    wT_nxt = build_wT(ot + 2)

    nc.compile()
    return nc


_nc_cache = {}


def _get_nc(key, **kw):
    if key not in _nc_cache:
        _nc_cache[key] = build_program(**kw)
    return _nc_cache[key]


def kernel(x: np.ndarray, sign_weights: np.ndarray, scales: np.ndarray) -> np.ndarray:
    nc = _get_nc("full")
    M_SH = M_FULL // N_CORES
    xf = np.ascontiguousarray(x.reshape(M_FULL, IN_F).astype(np.float32, copy=False))
    sw = np.ascontiguousarray(sign_weights.astype(np.float32, copy=False))
    sc = np.ascontiguousarray(scales.reshape(OUT_F, IN_F // GROUP))
    in_maps = [
        {"x": xf[c * M_SH:(c + 1) * M_SH], "sw": sw, "sc": sc}
        for c in range(N_CORES)
    ]
    res = run_bass_kernel_spmd(nc, in_maps, core_ids=list(range(N_CORES)))
    y = np.concatenate([res.results[c]["y"] for c in range(N_CORES)], axis=0)
    return y.reshape(B, S, OUT_F)
